# revision 28
# baseline (speedup 1.0000x reference)
"""Trainium2 Bass kernel for BoW: embedding gather + ragged segment-sum + Linear + ReLU.

Strategy (8 NeuronCores, data-parallel over sentences):
  - Core c owns segments [c*2048, (c+1)*2048). Tokens are split at sentence
    boundaries (segment_ids is sorted), so no cross-core reduction is needed.
  - Per core, tokens are grouped by (segment-window, vocab-shard):
      * segment-window: PSUM accumulator window of WIN segments (matmul target)
      * vocab-shard: dma_gather uses int16 indices, so the 100k-row table is
        addressed in shards of 32768 rows
    Groups are padded to a uniform size across all 8 cores so a single SPMD
    program serves every core (pad tokens gather row 0 and carry segment -1,
    which never matches the one-hot compare, so they contribute zero).
  - Embedding rows are fetched with dma_gather (bulk indirect DMA). For each
    128-token chunk, a one-hot matrix onehot[t, s] = (seg[t] == s) is built on
    VectorE via is_equal against an iota row, and TensorE accumulates
      bowT[d, s] += emb[t, d]^T @ onehot[t, s]
    into PSUM. Finally each 128-segment tile is multiplied by W, bias-added,
    ReLU'd, and DMA'd out.

Tuning (HW-measured, Aug 2026): the kernel is bound by the dma_gather
descriptor stream — ~1.8-2.3ns per gathered row, limited by per-queue
concurrency (4 SWDGE queues, the ucode max, round-robined) against HBM
latency.  bfloat16 table rows (256B descriptors) beat float32 by ~1.25x and
cost 1.8e-3 rel err (tolerance 2e-2).  Gathers are chopped to 896 indices so
each stays single_packet (<=1008 = 63 descs/lane, coalesced); win=256 keeps
one-hot integers exactly representable in bf16 and minimizes padding (4.1%).
Pool-engine SWDGE generation (~1us/gather fixed) and all of PE/DVE/ACT hide
under the gather.  idx/segf constant loads are sliced per window so the
first gather waits only on the first slice.  Compute exposure over a
gather-only ablation is ~10-20us; total ~207us vs the 371us f32 baseline.
"""

import numpy as np

N_CORES = 8
NSEG_TOTAL = 16384
SHARD_ROWS = 32768  # int16 gather index range
CHUNK = 128
# Pad slots as idx -1 requires num_idxs_reg = per-core valid count (the HW
# faults otherwise) — a Pool-engine register load per gather that costs about
# what the skipped pad descriptors save. Disabled; pads fetch row 0 instead.
PAD_SKIP = False


def _lane_slot(rank, gsize, gmax):
    """Within-group slot for within-group `rank` under lane-blocked order.

    Descriptor for slot s of a sub-gather is generated on ring lane s%16 and
    drained by SDMA engine s%16.  The default (rank == slot) order stripes
    consecutive vocab-sorted tokens across all 16 engines, so each engine's
    descriptor stream walks the table with a ~16x stride — every read opens
    a new DRAM row.  Lane-blocking gives each engine a CONTIGUOUS sorted run
    per sub-gather: slot = (r % (n/16))*16 + r//(n/16), so engine l serves
    sorted tokens [l*n/16, (l+1)*n/16).
    """
    span = rank // gmax
    r = rank - span * gmax
    n = np.minimum(gmax, gsize - span * gmax)
    per = n // 16
    return span * gmax + (r % per) * 16 + r // per


def _prep_host(tokens, segment_ids, vocab, nseg, win, shard_rows,
               gmax=None, laneblock=False, dedup=False):
    """Group tokens per core by (segment window, vocab shard); pad to uniform sizes.

    Returns per-core streams (int16 gather indices, f32 window-relative segment
    ids) plus the shared group size table.

    With dedup=True, runs of identical tokens within a group are merged
    pairwise: a merged slot gathers the row ONCE and carries TWO segment ids
    (segfa, segfb); the one-hot becomes is_equal(a) + is_equal(b), so a pair
    in the same segment correctly contributes 2.  This cuts gather
    descriptors by the within-group duplicate rate (~6%).
    """
    seg_per_core = nseg // N_CORES
    nst = seg_per_core // win
    n_shards = (vocab + shard_rows - 1) // shard_rows
    ngroups = nst * n_shards

    bounds = np.searchsorted(segment_ids, np.arange(N_CORES + 1) * seg_per_core)
    per_core = []
    counts = np.zeros((N_CORES, ngroups), dtype=np.int64)
    for c in range(N_CORES):
        lo, hi = bounds[c], bounds[c + 1]
        tok = tokens[lo:hi].astype(np.int64)
        seg = segment_ids[lo:hi].astype(np.int64) - c * seg_per_core
        st = seg // win
        sh = tok // shard_rows
        key = st * n_shards + sh
        n = tok.shape[0]
        # sort by (group, token id): token-sorted gathers walk the table
        # near-monotonically, and duplicates become adjacent
        order = np.lexsort((tok, key))
        tok_s, seg_s, key_s = tok[order], seg[order], key[order]
        if dedup and n:
            new_run = np.ones(n, dtype=bool)
            new_run[1:] = (tok_s[1:] != tok_s[:-1]) | (key_s[1:] != key_s[:-1])
            run_start_idx = np.nonzero(new_run)[0]
            run_id = np.cumsum(new_run) - 1
            pos = np.arange(n) - run_start_idx[run_id]
            keep = pos % 2 == 0
            ki = np.nonzero(keep)[0]
            has_b = np.zeros(ki.shape[0], dtype=bool)
            has_b[:-1] = (ki[:-1] + 1 < n) & (run_id[ki[:-1] + 1] == run_id[ki[:-1]])
            if ki.size:
                last = ki[-1]
                has_b[-1] = (last + 1 < n) and (run_id[last + 1] == run_id[last])
            sega = seg_s[ki]
            segb = np.where(has_b, seg_s[np.minimum(ki + 1, n - 1)], -1)
            tok_s, key_s = tok_s[ki], key_s[ki]
        else:
            sega = seg_s
            segb = None
        counts[c] = np.bincount(key_s, minlength=ngroups)
        per_core.append((tok_s, sega, segb, key_s))

    G = counts.max(axis=0)
    G = ((G + CHUNK - 1) // CHUNK) * CHUNK
    # Ensure every segment window gets at least one chunk so its PSUM region
    # is written (all-pad chunk writes zeros, which is the correct sum).
    for st in range(nst):
        if G[st * n_shards : (st + 1) * n_shards].sum() == 0:
            G[st * n_shards] = CHUNK
    off = np.concatenate([[0], np.cumsum(G)])
    tot = int(off[-1])

    # Pad slots gather idx -1: trailing negative indices are skipped by the
    # DMA (no fetch). The first 8 emitted groups keep idx 0 (fetch row 0) so
    # each emb pool buffer's first use writes finite data to all slots —
    # stale SBUF reuse afterwards only ever contains real table rows.
    nz = np.nonzero(G > 0)[0]
    first_fetch = set(nz[:8].tolist())

    idx_hw, segf_hw, segb_hw = [], [], []
    for c in range(N_CORES):
        tok_s, sega, segb, key_s = per_core[c]
        m = tok_s.shape[0]
        if PAD_SKIP:
            idx_stream = np.full(tot, -1, dtype=np.int16)
            for j in first_fetch:
                idx_stream[off[j] : off[j] + G[j]] = 0
        else:
            idx_stream = np.zeros(tot, dtype=np.int16)
        segf_stream = np.full(tot, -1.0, dtype=np.float32)
        segb_stream = np.full(tot, -1.0, dtype=np.float32)
        group_start = np.searchsorted(key_s, np.arange(ngroups))
        rank = np.arange(m) - group_start[key_s]
        if laneblock:
            gm = tot if gmax is None else (gmax // CHUNK) * CHUNK
            rank = _lane_slot(rank, G[key_s], gm)
        dest = off[key_s] + rank
        stwin = (key_s // n_shards) * win
        idx_stream[dest] = (tok_s % shard_rows).astype(np.int16)
        segf_stream[dest] = (sega - stwin).astype(np.float32)
        if segb is not None:
            segb_stream[dest] = np.where(
                segb >= 0, segb - stwin, -1
            ).astype(np.float32)
        # wrap by 16 partitions, replicate for the 8 gpsimd cores
        idx16 = np.tile(
            np.ascontiguousarray(idx_stream.reshape(tot // 16, 16).T), (8, 1)
        )
        idx_hw.append(np.ascontiguousarray(idx16))
        segf_hw.append(
            np.ascontiguousarray(segf_stream.reshape(tot // CHUNK, CHUNK).T)
        )
        segb_hw.append(
            np.ascontiguousarray(segb_stream.reshape(tot // CHUNK, CHUNK).T)
        )

    if not dedup:
        segb_hw = None
    return idx_hw, segf_hw, segb_hw, G, off, tot, nst, n_shards


LAST_RESULT = None  # BassKernelResults of the most recent run (for profiling)
LAST_NC = None
LAST_IN_MAPS = None
LAST_BUILD_ARGS = None  # (args, kwargs) to rebuild the program with reps=R


def _prep_host_stream(tokens, segment_ids, vocab, nseg, win, shard_rows,
                      gmax=896):
    """Stream layout: tokens grouped (shard-major, then window), padded to
    CHUNK per group, then the per-shard contiguous stream is chopped into
    uniform <=gmax single-packet gathers INDEPENDENT of group boundaries.
    This caps the dma_gather instruction count (994ns fixed SWDGE cost each)
    at ceil(tot/gmax) instead of ~2-3 per (window, shard) group.

    Returns per-core hw streams + the shared gather plan and per-chunk
    (window, start, stop) metadata.
    """
    seg_per_core = nseg // N_CORES
    nst = seg_per_core // win
    n_shards = (vocab + shard_rows - 1) // shard_rows
    # big shard 0 goes LAST in the stream so the 16 window-closings (and the
    # inline tails they trigger) spread across a long trailing segment
    sh_order = list(range(1, n_shards)) + [0]
    pos_of_sh = np.empty(n_shards, dtype=np.int64)
    for p, s in enumerate(sh_order):
        pos_of_sh[s] = p
    ngroups = n_shards * nst

    bounds = np.searchsorted(segment_ids, np.arange(N_CORES + 1) * seg_per_core)
    per_core = []
    counts = np.zeros((N_CORES, ngroups), dtype=np.int64)
    for c in range(N_CORES):
        lo, hi = bounds[c], bounds[c + 1]
        tok = tokens[lo:hi].astype(np.int64)
        seg = segment_ids[lo:hi].astype(np.int64) - c * seg_per_core
        st = seg // win
        sh = tok // shard_rows
        key = pos_of_sh[sh] * nst + st
        counts[c] = np.bincount(key, minlength=ngroups)
        per_core.append((tok, seg, st, key))

    G = counts.max(axis=0)
    G = ((G + CHUNK - 1) // CHUNK) * CHUNK
    # every window needs >=1 chunk so its PSUM region gets written
    for st in range(nst):
        if G[st::nst].sum() == 0:
            G[st] = CHUNK
    off = np.concatenate([[0], np.cumsum(G)])
    tot = int(off[-1])
    nchunks = tot // CHUNK

    # per-chunk metadata (uniform across cores): window id + start/stop flags
    chunk_st = np.empty(nchunks, dtype=np.int64)
    for g in range(ngroups):
        chunk_st[off[g] // CHUNK : off[g + 1] // CHUNK] = g % nst
    chunk_start = np.zeros(nchunks, dtype=bool)
    chunk_stop = np.zeros(nchunks, dtype=bool)
    for st in range(nst):
        idxs = np.nonzero(chunk_st == st)[0]
        chunk_start[idxs[0]] = True
        chunk_stop[idxs[-1]] = True

    # gather plan: chop each shard's contiguous stream span into <=gmax
    # chunks-aligned gathers
    plan = []
    for p, sh in enumerate(sh_order):
        lo = int(off[p * nst])
        hi = int(off[(p + 1) * nst])
        for o0 in range(lo, hi, gmax):
            n = min(gmax, hi - o0)
            if n > 0:
                plan.append((o0, n, sh * shard_rows,
                             min(shard_rows, vocab - sh * shard_rows)))

    idx_hw, segf_hw = [], []
    for c in range(N_CORES):
        tok, seg, st, key = per_core[c]
        n = tok.shape[0]
        idx_stream = np.zeros(tot, dtype=np.int16)
        segf_stream = np.full(tot, -1.0, dtype=np.float32)
        # token-sorted within each group: near-monotone HBM walk
        order = np.lexsort((tok, key))
        key_sorted = key[order]
        group_start = np.searchsorted(key_sorted, np.arange(ngroups))
        rank = np.arange(n) - group_start[key_sorted]
        dest = off[key_sorted] + rank
        idx_stream[dest] = (tok[order] % shard_rows).astype(np.int16)
        segf_stream[dest] = (seg[order] - st[order] * win).astype(np.float32)
        idx16 = np.tile(
            np.ascontiguousarray(idx_stream.reshape(tot // 16, 16).T), (8, 1)
        )
        segf = np.ascontiguousarray(segf_stream.reshape(nchunks, CHUNK).T)
        idx_hw.append(np.ascontiguousarray(idx16))
        segf_hw.append(segf)

    return (idx_hw, segf_hw, plan, chunk_st, chunk_start, chunk_stop, tot, nst,
            n_shards)


def _prep_host_wstream(tokens, segment_ids, vocab, nseg, win, shard_rows,
                       gmax=896, wrap=False):
    """Window-major stream layout with per-gather dynamic table base.

    Tokens are grouped by window only (no vocab shards) and token-sorted
    within each window. The stream is chopped into <=gmax single-packet
    gathers; each gather g reads tbl[B_g : B_g+32768] where B_g is the min
    token over all cores in that slice — sorted order makes every slice span
    far less than the int16 index range. Windows are strictly sequential in
    the stream, so PSUM accumulation groups never interleave.
    """
    seg_per_core = nseg // N_CORES
    nst = seg_per_core // win

    bounds = np.searchsorted(segment_ids, np.arange(N_CORES + 1) * seg_per_core)
    per_core = []
    counts = np.zeros((N_CORES, nst), dtype=np.int64)
    for c in range(N_CORES):
        lo, hi = bounds[c], bounds[c + 1]
        tok = tokens[lo:hi].astype(np.int64)
        seg = segment_ids[lo:hi].astype(np.int64) - c * seg_per_core
        st = seg // win
        counts[c] = np.bincount(st, minlength=nst)
        per_core.append((tok, seg, st))

    G = counts.max(axis=0)
    G = np.maximum(((G + CHUNK - 1) // CHUNK) * CHUNK, CHUNK)
    off = np.concatenate([[0], np.cumsum(G)])
    tot = int(off[-1])
    nchunks = tot // CHUNK

    chunk_st = np.empty(nchunks, dtype=np.int64)
    chunk_start = np.zeros(nchunks, dtype=bool)
    chunk_stop = np.zeros(nchunks, dtype=bool)
    for st in range(nst):
        c0, c1 = off[st] // CHUNK, off[st + 1] // CHUNK
        chunk_st[c0:c1] = st
        chunk_start[c0] = True
        chunk_stop[c1 - 1] = True

    # per-core sorted streams: token id per slot (-1 for pads) + segf
    tokstreams = []
    segf_hw = []
    for c in range(N_CORES):
        tok, seg, st = per_core[c]
        n = tok.shape[0]
        order = np.lexsort((tok, st))
        st_sorted = st[order]
        group_start = np.searchsorted(st_sorted, np.arange(nst))
        rank = np.arange(n) - group_start[st_sorted]
        dest = off[st_sorted] + rank
        tokstream = np.full(tot, -1, dtype=np.int64)
        tokstream[dest] = tok[order]
        segf_stream = np.full(tot, -1.0, dtype=np.float32)
        segf_stream[dest] = (seg[order] - st_sorted * win).astype(np.float32)
        tokstreams.append(tokstream)
        segf_hw.append(
            np.ascontiguousarray(segf_stream.reshape(nchunks, CHUNK).T)
        )

    # gather plan with per-gather table base. wrap=True chops the stream
    # continuously across window boundaries (fewer, uniform gathers); a
    # window-spanning gather uses a circular base against a table uploaded
    # with the first shard_rows rows appended again (table_rows below).
    plan = []
    base_per_slot = np.zeros(tot, dtype=np.int64)
    eff_streams = [ts.copy() for ts in tokstreams]
    if wrap:
        table_rows = vocab + shard_rows
        spans = [(0, tot)]
    else:
        table_rows = vocab
        spans = [(int(off[st]), int(off[st + 1])) for st in range(nst)]
    for lo, hi in spans:
        for o0 in range(lo, hi, gmax):
            n = min(gmax, hi - o0)
            st_lo = int(chunk_st[o0 // CHUNK])
            B, top = None, None
            for c in range(N_CORES):
                sl = eff_streams[c][o0 : o0 + n]
                if wrap:
                    # tokens of later windows get +vocab so the slice is
                    # monotone; stored back for idx computation below
                    head = chunk_st[(o0 + np.arange(n)) // CHUNK] > st_lo
                    sl = np.where((sl >= 0) & head & (sl < vocab),
                                  sl + vocab, sl)
                    eff_streams[c][o0 : o0 + n] = sl
                real = sl[sl >= 0]
                if real.size:
                    B = int(real.min()) if B is None else min(B, int(real.min()))
                    top = int(real.max()) if top is None else max(top, int(real.max()))
            if B is None:
                B, top = 0, 0
            assert top - B < shard_rows, (o0, B, top)
            base_per_slot[o0 : o0 + n] = B  # idx is relative to this
            if B >= vocab:
                B -= vocab  # whole slice lies in the appended copy's range
            rows = min(shard_rows, table_rows - B)
            plan.append((o0, n, B, rows))

    idx_hw = []
    for c in range(N_CORES):
        ts = eff_streams[c]
        idx = np.where(ts >= 0, ts - base_per_slot, 0)
        assert (idx >= 0).all() and (idx < shard_rows).all()
        idx_stream = idx.astype(np.int16)
        idx16 = np.tile(
            np.ascontiguousarray(idx_stream.reshape(tot // 16, 16).T), (8, 1)
        )
        idx_hw.append(np.ascontiguousarray(idx16))

    return (idx_hw, segf_hw, plan, chunk_st, chunk_start, chunk_stop, tot, nst,
            table_rows)


def _build_program_stream(plan, chunk_st, chunk_start, chunk_stop, tot, nst,
                          n_shards, win, nseg, mm_dtype_name, shard_rows,
                          vocab, dim, reps=1, parts="all", n_queues=4,
                          gmax=896, tail_inline=True, ebufs=8, obufs=4):
    """Stream-layout SPMD program: uniform single-packet gathers chopped
    independently of (window, shard) groups; per-chunk matmuls accumulate
    into each window's PSUM slice with host-tracked start/stop flags."""
    import concourse.bacc as bacc
    import concourse.mybir as mybir
    from concourse.tile import TileContext

    f32 = mybir.dt.float32
    i16 = mybir.dt.int16
    mm_dt = getattr(mybir.dt, mm_dtype_name)
    tbl_dt = mm_dt
    seg_per_core = nseg // N_CORES
    nchunks = tot // CHUNK
    gblk = gmax // CHUNK

    nc = bacc.Bacc("TRN2", num_devices=N_CORES, num_swdge_queues=n_queues)
    tbl_d = nc.declare_dram_parameter("tbl", [vocab, dim], tbl_dt, isOutput=False)
    idx_d = nc.declare_dram_parameter("idx", [128, tot // 16], i16, isOutput=False)
    segf_d = nc.declare_dram_parameter("segf", [128, nchunks], f32, isOutput=False)
    iota_d = nc.declare_dram_parameter("iota", [128, win], f32, isOutput=False)
    w_d = nc.declare_dram_parameter("w", [dim, dim], f32, isOutput=False)
    brep_d = nc.declare_dram_parameter("brep", [128, dim], f32, isOutput=False)
    out_d = nc.declare_dram_parameter("out", [seg_per_core, dim], f32, isOutput=True)

    with TileContext(nc) as tc:
        with (
            tc.tile_pool(name="const", bufs=1) as cpool,
            tc.tile_pool(name="emb", bufs=ebufs) as epool,
            tc.tile_pool(name="oh", bufs=obufs) as ohpool,
            tc.tile_pool(name="bow", bufs=1, space="PSUM") as bowpool,
            tc.tile_pool(name="o2", bufs=2, space="PSUM") as o2pool,
            tc.tile_pool(name="tail", bufs=3) as tailpool,
        ):
            idx_sb = cpool.tile([128, tot // 16], i16)
            nc.sync.dma_start(out=idx_sb[:], in_=idx_d[:])
            segf_sb = cpool.tile([128, nchunks], mm_dt)
            if mm_dt == f32:
                nc.sync.dma_start(out=segf_sb[:], in_=segf_d[:])
            else:
                segf_f32 = cpool.tile([128, nchunks], f32)
                nc.sync.dma_start(out=segf_f32[:], in_=segf_d[:])
                nc.vector.tensor_copy(out=segf_sb[:], in_=segf_f32[:])
            iota_sb = cpool.tile([128, win], mm_dt)
            if mm_dt == f32:
                nc.sync.dma_start(out=iota_sb[:], in_=iota_d[:])
            else:
                iota_f32 = cpool.tile([128, win], f32)
                nc.sync.dma_start(out=iota_f32[:], in_=iota_d[:])
                nc.vector.tensor_copy(out=iota_sb[:], in_=iota_f32[:])
            w_sb = cpool.tile([dim, dim], f32)
            nc.sync.dma_start(out=w_sb[:], in_=w_d[:])
            brep_sb = cpool.tile([128, dim], f32)
            nc.sync.dma_start(out=brep_sb[:], in_=brep_d[:])

            bow = None
            if parts != "gather":
                bow = bowpool.tile([128, seg_per_core], f32, tag="bow")
            dummy_emb = None
            if parts == "compute":
                dummy_emb = cpool.tile([128, dim], mm_dt, tag="dummy_emb")
                nc.vector.memset(dummy_emb[:], 0.0)
                tbl_touch = cpool.tile([128, dim], tbl_dt, tag="tbl_touch")
                nc.sync.dma_start(out=tbl_touch[:], in_=tbl_d[0:128, :])

            _gq = [0]

            def emit_tail(ot):
                bsb = tailpool.tile([128, 128], f32, tag="bsb")
                nc.vector.tensor_copy(
                    out=bsb[:], in_=bow[:, ot * 128 : (ot + 1) * 128]
                )
                o2 = o2pool.tile([128, dim], f32, tag="o2")
                nc.tensor.matmul(
                    out=o2[:], lhsT=bsb[:], rhs=w_sb[:], start=True, stop=True
                )
                osb = tailpool.tile([128, dim], f32, tag="osb")
                nc.vector.tensor_tensor(
                    out=osb[:], in0=o2[:], in1=brep_sb[:], op=mybir.AluOpType.add
                )
                nc.scalar.activation(
                    out=osb[:], in_=osb[:], func=mybir.ActivationFunctionType.Relu
                )
                nc.sync.dma_start(
                    out=out_d[ot * 128 : (ot + 1) * 128, :], in_=osb[:]
                )

            def emit_body():
                for (o0, n, base, rows) in plan:
                    nb = n // CHUNK
                    c0 = o0 // CHUNK
                    if parts in ("all", "gather"):
                        emb = epool.tile([128, gblk, dim], tbl_dt, tag="emb")
                        nc.gpsimd.dma_gather(
                            emb[:, :nb, :],
                            tbl_d[base : base + rows, :],
                            idx_sb[:, o0 // 16 : (o0 + n) // 16],
                            num_idxs=n,
                            num_idxs_reg=n,
                            elem_size=dim,
                            single_packet=(n <= 1008),
                            queue_num=_gq[0] % n_queues,
                        )
                        _gq[0] += 1
                    if parts == "gather":
                        continue
                    ohg = ohpool.tile([128, gblk, win], mm_dt, tag="oh")
                    seg_b = segf_sb[:, c0 : c0 + nb].broadcast_to([128, nb, win])
                    iota_b = iota_sb[:].rearrange(
                        "p (a w) -> p a w", a=1
                    ).broadcast_to([128, nb, win])
                    nc.vector.tensor_tensor(
                        out=ohg[:, :nb, :],
                        in0=iota_b,
                        in1=seg_b,
                        op=mybir.AluOpType.is_equal,
                    )
                    for li in range(nb):
                        c = c0 + li
                        stc = int(chunk_st[c])
                        lhsT = (
                            dummy_emb[:] if parts == "compute"
                            else emb[:, li, :]
                        )
                        nc.tensor.matmul(
                            out=bow[:, stc * win : (stc + 1) * win],
                            lhsT=lhsT,
                            rhs=ohg[:, li, :],
                            start=bool(chunk_start[c]),
                            stop=bool(chunk_stop[c]),
                            skip_group_check=True,
                        )
                        if chunk_stop[c] and tail_inline:
                            for ot in range(stc * win // 128,
                                            (stc + 1) * win // 128):
                                emit_tail(ot)
                if parts != "gather" and not tail_inline:
                    for ot in range(seg_per_core // 128):
                        emit_tail(ot)

            for _ in range(reps):
                emit_body()

    nc.compile()
    return nc


def _build_program(G, off, tot, nst, n_shards, win, nseg, mm_dtype_name,
                   shard_rows, vocab, dim, reps=1, parts="all", n_queues=1,
                   gmax=None, tail_inline=True, ebufs=5, obufs=3, dedup=False):
    """Build the (core-uniform) SPMD Bass program. Returns the compiled nc."""
    import concourse.bacc as bacc
    import concourse.mybir as mybir
    from concourse.tile import TileContext

    f32 = mybir.dt.float32
    i16 = mybir.dt.int16
    # "mixed": gather f32 rows (512B descriptors run ~2x faster than 256B),
    # cast to bf16 on-chip, run bf16 matmuls. "bf16oh": gather float32r rows
    # and keep them as the stationary matmul operand (no cast); only the
    # one-hot (the moving operand, which sets the PE cycles/row) is bf16.
    # Otherwise table dtype == compute dtype. float32r tiles must be declared
    # as such (verifier wants rounded producers).
    cast_emb = False
    if mm_dtype_name == "mixed":
        mm_dt = mybir.dt.bfloat16
        tbl_dt = f32
        cast_emb = True
    elif mm_dtype_name == "bf16oh":
        mm_dt = mybir.dt.bfloat16
        tbl_dt = mybir.dt.float32r
    else:
        mm_dt = getattr(mybir.dt, mm_dtype_name)
        tbl_dt = mm_dt
    seg_per_core = nseg // N_CORES
    nchunks = tot // CHUNK

    # seg2x: segf is duplicated pairwise on the host so the one-hot
    # is_equal's innermost AP dim is a stride-1 pair of 2-byte elements —
    # the DVE's 2x (dual-pump 16-bit) mode requires packed innermost
    # elements on EVERY operand; a stride-0 broadcast innermost dim forces
    # 1x throughput.
    seg2x = mm_dtype_name == "bfloat16"
    segw = 2 * nchunks if seg2x else nchunks

    nc = bacc.Bacc("TRN2", num_devices=N_CORES, num_swdge_queues=n_queues)
    tbl_d = nc.declare_dram_parameter("tbl", [vocab, dim], tbl_dt, isOutput=False)
    idx_d = nc.declare_dram_parameter("idx", [128, tot // 16], i16, isOutput=False)
    segf_d = nc.declare_dram_parameter("segf", [128, segw], f32, isOutput=False)
    segb_d = (nc.declare_dram_parameter("segb", [128, segw], f32, isOutput=False)
              if dedup else None)
    iota_d = nc.declare_dram_parameter("iota", [128, win], f32, isOutput=False)
    w_d = nc.declare_dram_parameter("w", [dim, dim], f32, isOutput=False)
    brep_d = nc.declare_dram_parameter("brep", [128, dim], f32, isOutput=False)
    out_d = nc.declare_dram_parameter("out", [seg_per_core, dim], f32, isOutput=True)

    with TileContext(nc) as tc:
        with (
            tc.tile_pool(name="const", bufs=1) as cpool,
            tc.tile_pool(name="emb", bufs=ebufs) as epool,
            tc.tile_pool(name="oh", bufs=obufs) as ohpool,
            tc.tile_pool(name="bow", bufs=1, space="PSUM") as bowpool,
            tc.tile_pool(name="o2", bufs=2, space="PSUM") as o2pool,
            tc.tile_pool(name="tail", bufs=3) as tailpool,
        ):
            iota_sb = cpool.tile([128, win], mm_dt)
            if mm_dt == f32:
                nc.sync.dma_start(out=iota_sb[:], in_=iota_d[:])
            else:
                iota_f32 = cpool.tile([128, win], f32)
                nc.sync.dma_start(out=iota_f32[:], in_=iota_d[:])
                nc.vector.tensor_copy(out=iota_sb[:], in_=iota_f32[:])
            w_sb = cpool.tile([dim, dim], f32)
            nc.sync.dma_start(out=w_sb[:], in_=w_d[:])
            brep_sb = cpool.tile([128, dim], f32)
            nc.sync.dma_start(out=brep_sb[:], in_=brep_d[:])
            # idx/segf loads sliced per window: the first gather then waits
            # only on window 0's slice (~1/nst of the bytes), not the whole
            # 1.7MB stream — trims the serial startup head.
            idx_sb = cpool.tile([128, tot // 16], i16)
            segf_sb = cpool.tile([128, segw], mm_dt)
            segb_sb = None
            if dedup:
                segb_sb = cpool.tile([128, segw], mm_dt, tag="segb_sb")
            segf_f32 = None
            segb_f32 = None
            if mm_dt != f32:
                segf_f32 = cpool.tile([128, segw], f32)
                if dedup:
                    segb_f32 = cpool.tile([128, segw], f32, tag="segb_f32")
            segmul = segw // nchunks  # 2 when seg2x else 1
            woff = np.concatenate([[0], np.cumsum(
                [int(G[st * n_shards : (st + 1) * n_shards].sum())
                 for st in range(nst)])])

            def load_seg(dst, stage, src, ca, cb):
                if mm_dt == f32:
                    nc.sync.dma_start(out=dst[:, ca:cb], in_=src[:, ca:cb])
                else:
                    nc.sync.dma_start(out=stage[:, ca:cb], in_=src[:, ca:cb])
                    nc.vector.tensor_copy(out=dst[:, ca:cb],
                                          in_=stage[:, ca:cb])

            for st in range(nst):
                a, b_ = int(woff[st]), int(woff[st + 1])
                if b_ == a:
                    continue
                nc.sync.dma_start(out=idx_sb[:, a // 16 : b_ // 16],
                                  in_=idx_d[:, a // 16 : b_ // 16])
                ca, cb = segmul * (a // CHUNK), segmul * (b_ // CHUNK)
                load_seg(segf_sb, segf_f32, segf_d, ca, cb)
                if dedup:
                    load_seg(segb_sb, segb_f32, segb_d, ca, cb)

            bow = None
            if parts != "gather":
                bow = bowpool.tile([128, seg_per_core], f32, tag="bow")
            dummy_emb = None
            if parts == "compute":
                dummy_emb = cpool.tile([128, dim], mm_dt, tag="dummy_emb")
                nc.vector.memset(dummy_emb[:], 0.0)
                # keep tbl referenced so walrus doesn't see a userless dram tensor
                tbl_touch = cpool.tile([128, dim], tbl_dt, tag="tbl_touch")
                nc.sync.dma_start(out=tbl_touch[:], in_=tbl_d[0:128, :])

            max_blk = int(G.max()) // CHUNK

            _gq = [0]  # gather counter for queue round-robin

            def emit_tail(ot):
                bsb = tailpool.tile([128, 128], f32, tag="bsb")
                nc.vector.tensor_copy(
                    out=bsb[:], in_=bow[:, ot * 128 : (ot + 1) * 128]
                )
                o2 = o2pool.tile([128, dim], f32, tag="o2")
                nc.tensor.matmul(
                    out=o2[:], lhsT=bsb[:], rhs=w_sb[:], start=True, stop=True
                )
                osb = tailpool.tile([128, dim], f32, tag="osb")
                nc.vector.tensor_tensor(
                    out=osb[:], in0=o2[:], in1=brep_sb[:], op=mybir.AluOpType.add
                )
                nc.scalar.activation(
                    out=osb[:], in_=osb[:], func=mybir.ActivationFunctionType.Relu
                )
                nc.sync.dma_start(
                    out=out_d[ot * 128 : (ot + 1) * 128, :], in_=osb[:]
                )

            def emit_body():
                gc = 0
                for st in range(nst):
                    first_mm_of_win = True
                    # last group index in this window with G>0
                    live = [s for s in range(n_shards) if G[st * n_shards + s] > 0]
                    for sh in range(n_shards):
                        j = st * n_shards + sh
                        gj = int(G[j])
                        if gj == 0:
                            continue
                        nblk = gj // CHUNK
                        emb = None
                        if parts != "compute":
                            emb = epool.tile([128, max_blk, dim], tbl_dt, tag="emb")
                        rows = min(shard_rows, vocab - sh * shard_rows)
                        if parts in ("all", "gather"):
                            step = nblk if gmax is None else max(1, gmax // CHUNK)
                            for sub in range(0, nblk, step):
                                k = min(step, nblk - sub)
                                sgj = k * CHUNK
                                o0 = int(off[j]) + sub * CHUNK
                                nc.gpsimd.dma_gather(
                                    emb[:, sub : sub + k, :],
                                    tbl_d[
                                        sh * shard_rows : sh * shard_rows + rows, :
                                    ],
                                    idx_sb[:, o0 // 16 : (o0 + sgj) // 16],
                                    num_idxs=sgj,
                                    num_idxs_reg=sgj,
                                    elem_size=dim,
                                    single_packet=(sgj <= 1008),
                                    queue_num=_gq[0] % n_queues,
                                )
                                _gq[0] += 1
                        if parts == "gather":
                            continue
                        mm_emb = emb
                        if cast_emb and parts != "compute":
                            # cast gathered rows f32 -> bf16; alternate DVE/ACT
                            # so the cast stays off whichever engine is busier
                            mm_emb = epool.tile(
                                [128, max_blk, dim], mm_dt, tag="emb16"
                            )
                            nc.scalar.copy(
                                out=mm_emb[:, :nblk, :], in_=emb[:, :nblk, :]
                            )
                        # one-hot in sub-group batches: oh[t, c, s] =
                        # (segf[t, gc+c] == iota[s]). Smaller tiles than
                        # one-per-group keep SBUF free for gather buffers.
                        oh_blk = min(max_blk, 10)
                        for ob in range(0, nblk, oh_blk):
                            kb = min(oh_blk, nblk - ob)
                            ohg = ohpool.tile([128, oh_blk, win], mm_dt, tag="oh")

                            def seg_ap(sb, a, b):
                                if seg2x:
                                    # [p, c, win/2, 2] with the pair innermost
                                    # on every operand: DVE 2x mode eligible.
                                    return sb[:, 2 * a : 2 * b].rearrange(
                                        "p (c two) -> p c two", two=2
                                    ).rearrange(
                                        "p c (h two) -> p c h two", h=1
                                    ).broadcast_to([128, b - a, win // 2, 2])
                                return sb[:, a:b].broadcast_to([128, b - a, win])

                            if seg2x:
                                iota_b = iota_sb[:].rearrange(
                                    "p (a h two) -> p a h two", a=1, two=2
                                ).broadcast_to([128, kb, win // 2, 2])
                                oh_out = ohg[:, :kb, :].rearrange(
                                    "p c (h two) -> p c h two", two=2
                                )
                            else:
                                iota_b = iota_sb[:].rearrange(
                                    "p (a w) -> p a w", a=1
                                ).broadcast_to([128, kb, win])
                                oh_out = ohg[:, :kb, :]
                            nc.vector.tensor_tensor(
                                out=oh_out,
                                in0=iota_b,
                                in1=seg_ap(segf_sb, gc + ob, gc + ob + kb),
                                op=mybir.AluOpType.is_equal,
                            )
                            if dedup:
                                # merged-pair second segment: one-hot becomes
                                # is_equal(a) + is_equal(b)  (2 if a == b)
                                ohb = ohpool.tile([128, oh_blk, win], mm_dt,
                                                  tag="ohb")
                                ohb_out = (ohb[:, :kb, :].rearrange(
                                    "p c (h two) -> p c h two", two=2
                                ) if seg2x else ohb[:, :kb, :])
                                nc.vector.tensor_tensor(
                                    out=ohb_out,
                                    in0=iota_b,
                                    in1=seg_ap(segb_sb, gc + ob, gc + ob + kb),
                                    op=mybir.AluOpType.is_equal,
                                )
                                nc.vector.tensor_tensor(
                                    out=ohg[:, :kb, :],
                                    in0=ohg[:, :kb, :],
                                    in1=ohb[:, :kb, :],
                                    op=mybir.AluOpType.add,
                                )
                            for cblk in range(ob, ob + kb):
                                is_last = sh == live[-1] and cblk == nblk - 1
                                lhsT = (
                                    dummy_emb[:] if parts == "compute"
                                    else mm_emb[:, cblk, :]
                                )
                                nc.tensor.matmul(
                                    out=bow[:, st * win : (st + 1) * win],
                                    lhsT=lhsT,
                                    rhs=ohg[:, cblk - ob, :],
                                    start=first_mm_of_win,
                                    stop=is_last,
                                )
                                first_mm_of_win = False
                        gc += nblk
                    if parts != "gather" and tail_inline:
                        # window st's PSUM region is closed — drain it to the
                        # output while later windows still gather
                        for ot in range(st * win // 128, (st + 1) * win // 128):
                            emit_tail(ot)

                if parts != "gather" and not tail_inline:
                    for ot in range(seg_per_core // 128):
                        emit_tail(ot)

            for _ in range(reps):
                emit_body()

    nc.compile()
    return nc


def _make_in_maps(idx_hw, segf_hw, table, W, b, win, mm_dtype_name,
                  wrap_rows=0, segb_hw=None):
    import concourse.mybir as mybir

    if mm_dtype_name == "mixed":
        tbl_dt = mybir.dt.float32
    elif mm_dtype_name == "bf16oh":
        tbl_dt = mybir.dt.float32r
    else:
        tbl_dt = getattr(mybir.dt, mm_dtype_name)
    iota_hw = np.tile(np.arange(win, dtype=np.float32), (128, 1))
    brep_hw = np.tile(b.astype(np.float32), (128, 1))
    if wrap_rows:
        table = np.concatenate([table, table[:wrap_rows]], axis=0)
    tbl_np = np.ascontiguousarray(table.astype(mybir.dt.np(tbl_dt)))
    # seg2x duplication for the group builder's DVE-2x one-hot (see
    # _build_program): segf[p, c] -> segf[p, 2c], segf[p, 2c+1].
    seg2x = mm_dtype_name == "bfloat16"

    def segmap(s):
        return (np.ascontiguousarray(np.repeat(s, 2, axis=1))
                if seg2x else s)

    maps = [
        {
            "tbl": tbl_np,
            "idx": idx_hw[c],
            "segf": segmap(segf_hw[c]),
            "iota": iota_hw,
            "w": np.ascontiguousarray(W.astype(np.float32)),
            "brep": brep_hw,
        }
        for c in range(N_CORES)
    ]
    if segb_hw is not None:
        for c in range(N_CORES):
            maps[c]["segb"] = segmap(segb_hw[c])
    return maps


def kernel(tokens, segment_ids, embedding_table, W, b, *, nseg=NSEG_TOTAL, win=256,
           mm_dtype="bfloat16", shard_rows=SHARD_ROWS, trace=False, n_queues=4,
           gmax=896, layout="group", tail_inline=True, ebufs=None, obufs=None,
           laneblock=False, dedup=False):
    from concourse.bass_utils import run_bass_kernel_spmd

    tokens = np.asarray(tokens, dtype=np.int32)
    segment_ids = np.asarray(segment_ids, dtype=np.int32)
    embedding_table = np.asarray(embedding_table, dtype=np.float32)
    W = np.asarray(W, dtype=np.float32)
    b = np.asarray(b, dtype=np.float32)
    vocab, dim = embedding_table.shape

    bufs_kw = {}
    if ebufs is not None:
        bufs_kw["ebufs"] = ebufs
    if obufs is None and dedup and layout == "group":
        # dedup allocates two oh-pool tiles per block (oh + ohb); double the
        # ring so the effective pipeline depth stays at 3 blocks
        bufs_kw["obufs"] = 6
    elif obufs is not None:
        bufs_kw["obufs"] = obufs
    if layout in ("wstream", "cstream"):
        wrap = layout == "cstream"
        (idx_hw, segf_hw, plan, chunk_st, chunk_start, chunk_stop, tot,
         nst, table_rows) = _prep_host_wstream(
            tokens, segment_ids, vocab, nseg, win, shard_rows, gmax, wrap=wrap
        )
        build_args = (plan, chunk_st, chunk_start, chunk_stop, tot, nst,
                      1, win, nseg, mm_dtype, shard_rows, table_rows, dim)
        build_kw = dict(n_queues=n_queues, gmax=gmax, tail_inline=tail_inline,
                        **bufs_kw)
        nc = _build_program_stream(*build_args, **build_kw)
    elif layout == "stream":
        (idx_hw, segf_hw, plan, chunk_st, chunk_start, chunk_stop, tot, nst,
         n_shards) = _prep_host_stream(
            tokens, segment_ids, vocab, nseg, win, shard_rows, gmax
        )
        build_args = (plan, chunk_st, chunk_start, chunk_stop, tot, nst,
                      n_shards, win, nseg, mm_dtype, shard_rows, vocab, dim)
        build_kw = dict(n_queues=n_queues, gmax=gmax, tail_inline=tail_inline,
                        **bufs_kw)
        nc = _build_program_stream(*build_args, **build_kw)
    else:
        idx_hw, segf_hw, segb_hw, G, off, tot, nst, n_shards = _prep_host(
            tokens, segment_ids, vocab, nseg, win, shard_rows,
            gmax=gmax, laneblock=laneblock, dedup=dedup
        )
        build_args = (G, off, tot, nst, n_shards, win, nseg, mm_dtype,
                      shard_rows, vocab, dim)
        build_kw = dict(n_queues=n_queues, gmax=gmax, tail_inline=tail_inline,
                        dedup=dedup, **bufs_kw)
        nc = _build_program(*build_args, **build_kw)
    wrap_rows = table_rows - vocab if layout == "cstream" else 0
    segb_kw = {}
    if layout not in ("stream", "wstream", "cstream") and dedup:
        segb_kw["segb_hw"] = segb_hw
    in_maps = _make_in_maps(idx_hw, segf_hw, embedding_table, W, b, win,
                            mm_dtype, wrap_rows=wrap_rows, **segb_kw)
    res = run_bass_kernel_spmd(
        nc, in_maps, core_ids=list(range(N_CORES)), trace=trace
    )
    global LAST_RESULT, LAST_NC, LAST_IN_MAPS, LAST_BUILD_ARGS
    LAST_RESULT = res
    LAST_NC = nc
    LAST_IN_MAPS = in_maps
    builder = (_build_program_stream if layout in ("stream", "wstream", "cstream")
               else _build_program)
    LAST_BUILD_ARGS = (builder, build_args, build_kw)
    return np.concatenate([res.results[c]["out"] for c in range(N_CORES)], axis=0)

